# revision 30
# baseline (speedup 1.0000x reference)
"""CapsNet forward pass on 8 Trainium2 NeuronCores (pure data parallelism).

Per core (4 images), all heavy math in fp8 DoubleRow on the PE:
  conv1 (256x1x30x160, stride (1,4)) as 20 accumulated fp8-DoubleRow matmuls
  per psum row (width-tap pair (a, a+20) per instruction -- the k-pair dim
  needs a multi-byte stride, a 1B stride hangs the real PE; K=(s,i)=120 of
  128 partitions x 2 k-tiles = 240 taps/step at 0.5 cyc/row); the Toeplitz
  operand is built on the HOST and shipped as one DMA per image.
  Relu+bias+fp8-quantize evac on ACT directly into the primary-conv
  stationary layout.
  -> primary caps conv (256x256x10x10, stride 10) as 100 fp8-DoubleRow
  matmuls (both 128-channel halves per instruction), output [spatial, ch].
  -> squash (+ fp8/bias scale folding) on DVE -> capsule prediction as
  per-partition-scalar MACs on DVE (walrus rejects TensorScalarPtr on Pool),
  bf16 accumulation.
  -> agreement routing with all contractions on the PE: per-(g,im) matmuls
  against a block-diagonal v operand for the b-updates, c-weighted capsule
  sums, and PE transposes for layout changes; only softmax/squash pointwise
  work stays on DVE/ACT (bf16).
  -> class head via a K=1 broadcast matmul + DVE contraction + free-dim
  log_softmax. Host only shards/stacks.
"""

import numpy as np
import ml_dtypes
from contextlib import ExitStack

import concourse.bass as bass
import concourse.tile as tile
import concourse.mybir as mybir
from concourse.bass_utils import run_bass_kernel_spmd  # noqa: F401  (kept for parity)

F32 = mybir.dt.float32
BF16 = mybir.dt.bfloat16
FP8 = mybir.dt.float8e4
AF = mybir.ActivationFunctionType
ALU = mybir.AluOpType
DR = mybir.MatmulPerfMode.DoubleRow

E4NP = ml_dtypes.float8_e4m3
BFNP = ml_dtypes.bfloat16

# Problem constants
N_CORES = 8
IMG = 4              # images per core
HI, WI = 120, 640    # input image
KH, KW = 30, 160     # conv1 kernel
S = 4                # conv1 width stride (phase count)
A = KW // S          # 40 width taps per phase
NAP = A // 2         # 20 DoubleRow tap-pair steps
HOU, WOU = 90, 120   # conv1 output rows/cols actually consumed by prim conv
HB = 4               # conv1 row-block (psum bank = 2 DoubleRow matmuls)
C1 = 256
PQ = 100             # prim kernel positions (10x10)
EF = 108             # prim output spatial (9*12)
D2 = 256             # prim output channels
G = 32               # capsule groups
NK = 8               # capsule input dim
OD = 80              # 5 classes * 16
NCL, FD = 5, 16
NITER = 4            # initial softmax round + 3 routing iterations
YM = 160             # toeplitz m extent
XP = 2 * PQ * EF     # x free pitch (21600)
WSC = 16.0           # fp8 weight scale for conv1/prim
CWS = 16.0           # fp8 scale for caps weights

DVE_G = 20           # capsule groups on DVE; rest on Pool(gpsimd)


def _emit(nc):
    import os
    stage = os.environ.get("K_STAGE", "all")  # dma|mm|conv|prim|caps|all
    # ---- DRAM I/O ----
    d_ydf = nc.dram_tensor("ydf", [IMG, 128, HOU, YM], FP8, kind="ExternalInput")
    d_w1c = nc.dram_tensor("w1c", [128, NAP, 2, C1], FP8, kind="ExternalInput")
    d_b1 = nc.dram_tensor("b1", [128, 2], F32, kind="ExternalInput")
    d_wpq = nc.dram_tensor("wpq", [128, 2, PQ, D2], FP8, kind="ExternalInput")
    d_bp16 = nc.dram_tensor("bp16", [D2], F32, kind="ExternalInput")
    d_cw = nc.dram_tensor("cw", [EF, G, NK, OD], FP8, kind="ExternalInput")
    d_bb0 = nc.dram_tensor("bb0", [EF, IMG, G, NCL], BF16, kind="ExternalInput")
    d_c0 = nc.dram_tensor("c0", [EF, G, NCL], BF16, kind="ExternalInput")
    d_mask = nc.dram_tensor("mask", [OD, NCL], BF16, kind="ExternalInput")
    d_idb = nc.dram_tensor("idb", [128, 128], BF16, kind="ExternalInput")
    d_idf = nc.dram_tensor("idf", [128, 128], F32, kind="ExternalInput")
    d_wcf = nc.dram_tensor("wcf", [1, 26 * FD + 26], F32, kind="ExternalInput")
    d_out = nc.dram_tensor("out", [IMG, NCL * 26], F32, kind="ExternalOutput")

    # ---- persistent SBUF ----
    t_ydf = [nc.alloc_sbuf_tensor(f"s_ydf{i}", [128, HOU, YM], FP8)
             for i in range(2)]
    t_w1c = nc.alloc_sbuf_tensor("s_w1c", [128, NAP, 2, C1], FP8)
    t_b1 = nc.alloc_sbuf_tensor("s_b1", [128, 2], F32)
    t_x = nc.alloc_sbuf_tensor("s_x", [128, 2, PQ, EF], FP8)
    t_wpq = nc.alloc_sbuf_tensor("s_wpq", [128, 2, PQ, D2], FP8)
    t_bpx = nc.alloc_sbuf_tensor("s_bpx", [EF, D2], F32)
    t_cw = nc.alloc_sbuf_tensor("s_cw", [EF, G, NK, OD], FP8)
    t_pb = nc.alloc_sbuf_tensor("s_pb", [EF, D2], F32)
    t_sq = nc.alloc_sbuf_tensor("s_sq", [EF, D2], F32)
    t_l2 = nc.alloc_sbuf_tensor("s_l2", [EF, G], F32)
    t_f1 = nc.alloc_sbuf_tensor("s_f1", [EF, G], F32)
    t_f2 = nc.alloc_sbuf_tensor("s_f2", [EF, G], F32)
    t_f3 = nc.alloc_sbuf_tensor("s_f3", [EF, G], F32)
    t_f4 = nc.alloc_sbuf_tensor("s_f4", [EF, G], F32)
    t_f5 = nc.alloc_sbuf_tensor("s_f5", [EF, G], F32)
    t_u = nc.alloc_sbuf_tensor("s_u", [EF, IMG, G, NK], F32)
    t_upA = nc.alloc_sbuf_tensor("s_upA", [EF, DVE_G, IMG, OD], BF16)
    t_upB = nc.alloc_sbuf_tensor("s_upB", [EF, G - DVE_G, IMG, OD], BF16)
    t_upT = nc.alloc_sbuf_tensor("s_upT", [OD, IMG, G, EF], BF16)
    t_bb = nc.alloc_sbuf_tensor("s_bb", [EF, IMG, G, NCL], BF16)
    t_ce = nc.alloc_sbuf_tensor("s_ce", [EF, IMG, G, NCL], BF16)
    t_cc = nc.alloc_sbuf_tensor("s_cc", [EF, IMG, G, NCL], BF16)
    t_cs = nc.alloc_sbuf_tensor("s_cs", [EF, IMG * G], F32)
    t_cr = nc.alloc_sbuf_tensor("s_cr", [EF, IMG * G], F32)
    t_c0 = nc.alloc_sbuf_tensor("s_c0", [EF, G, NCL], BF16)
    t_mask = nc.alloc_sbuf_tensor("s_mask", [OD, NCL], BF16)
    t_idb = nc.alloc_sbuf_tensor("s_idb", [128, 128], BF16)
    t_idf = nc.alloc_sbuf_tensor("s_idf", [128, 128], F32)
    t_sm = nc.alloc_sbuf_tensor("s_sm", [OD, IMG, NCL], F32)
    t_sT = nc.alloc_sbuf_tensor("s_sT", [OD, IMG], BF16)
    t_sf = nc.alloc_sbuf_tensor("s_sf", [IMG, OD], F32)
    t_q1 = nc.alloc_sbuf_tensor("s_q1", [IMG, OD], F32)
    t_q2 = nc.alloc_sbuf_tensor("s_q2", [IMG, NCL], F32)
    t_q3 = nc.alloc_sbuf_tensor("s_q3", [IMG, NCL], F32)
    t_q4 = nc.alloc_sbuf_tensor("s_q4", [IMG, NCL], F32)
    t_q5 = nc.alloc_sbuf_tensor("s_q5", [IMG, NCL], F32)
    t_q6 = nc.alloc_sbuf_tensor("s_q6", [IMG, NCL], F32)
    t_v = nc.alloc_sbuf_tensor("s_v", [IMG, OD], BF16)
    t_vf = nc.alloc_sbuf_tensor("s_vf", [IMG, OD], F32)
    t_vT = nc.alloc_sbuf_tensor("s_vT", [OD, IMG], BF16)
    t_vbd = nc.alloc_sbuf_tensor("s_vbd", [OD, IMG, NCL], BF16)
    t_o4 = nc.alloc_sbuf_tensor("s_o4", [1, IMG], F32)
    t_z8 = nc.alloc_sbuf_tensor("s_z8", [EF, OD], BF16)
    t_wcf = nc.alloc_sbuf_tensor("s_wcf", [1, 26 * FD + 26], F32)
    t_wcb = nc.alloc_sbuf_tensor("s_wcb", [IMG, 26 * FD + 26], F32)
    t_hm = nc.alloc_sbuf_tensor("s_hm", [IMG, 26 * FD], F32)
    t_li = nc.alloc_sbuf_tensor("s_li", [IMG, NCL, 26], F32)
    t_ee = nc.alloc_sbuf_tensor("s_ee", [IMG, NCL, 26], F32)
    t_es = nc.alloc_sbuf_tensor("s_es", [IMG, NCL], F32)
    t_ln = nc.alloc_sbuf_tensor("s_ln", [IMG, NCL], F32)
    t_lg = nc.alloc_sbuf_tensor("s_lg", [IMG, NCL * 26], F32)

    YP = HOU * YM          # ydf free pitch per partition (elements)

    def up_slice(g, im):
        t = t_upA if g < DVE_G else t_upB
        gg = g if g < DVE_G else g - DVE_G
        return t.ap()[:, gg, im, :]

    with tile.TileContext(nc) as tc, nc.allow_low_precision(reason="bf16 routing"):
        with ExitStack() as ctx:
            # bank budget: cps 2 tags x 2 bufs + pps 1 + tps 2 = 7 of 8
            cps = ctx.enter_context(tc.tile_pool(name="cps", bufs=2, space="PSUM"))
            pps = ctx.enter_context(tc.tile_pool(name="pps", bufs=1, space="PSUM"))
            tps = ctx.enter_context(tc.tile_pool(name="tps", bufs=2, space="PSUM"))

            # ---- constant loads (order matters: conv1(im0) deps first) ----
            nc.sync.dma_start(out=t_w1c.ap(), in_=d_w1c.ap())
            # ydf im0 in two chunks so conv1 can start early
            nc.sync.dma_start(out=t_ydf[0].ap()[:, 0:20, :], in_=d_ydf.ap()[0][:, 0:20, :])
            nc.sync.dma_start(out=t_b1.ap(), in_=d_b1.ap())
            nc.sync.dma_start(out=t_ydf[0].ap()[:, 20:HOU, :], in_=d_ydf.ap()[0][:, 20:HOU, :])
            nc.sync.dma_start(out=t_wpq.ap(), in_=d_wpq.ap())
            nc.sync.dma_start(
                out=t_bpx.ap(), in_=bass.AP(tensor=d_bp16, offset=0, ap=[[0, EF], [1, D2]])
            )
            nc.sync.dma_start(out=t_cw.ap(), in_=d_cw.ap())
            nc.sync.dma_start(out=t_bb.ap(), in_=d_bb0.ap())
            nc.sync.dma_start(out=t_c0.ap(), in_=d_c0.ap())
            nc.sync.dma_start(out=t_mask.ap(), in_=d_mask.ap())
            nc.sync.dma_start(out=t_idb.ap(), in_=d_idb.ap())
            nc.sync.dma_start(out=t_idf.ap(), in_=d_idf.ap())
            nc.sync.dma_start(out=t_wcf.ap(), in_=d_wcf.ap())
            nc.vector.memset(t_o4.ap(), 1.0)
            nc.vector.memset(t_lg.ap(), 0.0)
            nc.vector.memset(t_z8.ap(), 0.0)

            # ================= per-image pipeline =================
            for im in range(IMG):
                ydf = t_ydf[im % 2]
                if im + 1 < IMG:
                    nc.sync.dma_start(out=t_ydf[(im + 1) % 2].ap(),
                                      in_=d_ydf.ap()[im + 1])
                if stage == "dma":
                    continue

                # ---- conv1: 23 blocks of 4 rows (last: 2) ----
                nblk = (HOU + HB - 1) // HB
                for b in range(nblk):
                    h0 = b * HB
                    hb = min(HB, HOU - h0)
                    ps = [cps.tile([128, HB, WOU], F32, tag=f"c{c2}",
                                   name=f"ps{c2}")
                          for c2 in range(2)]
                    mm = os.environ.get("K_MM", "dr")
                    for ap_i in range(NAP):
                        for c2 in range(2):
                            lhsT = t_w1c.ap()[:, ap_i, :, c2 * 128:(c2 + 1) * 128]
                            for r in range(hb):
                                st_ = (ap_i == 0 and r == 0)
                                sp_ = (ap_i == NAP - 1 and r == hb - 1)
                                if mm == "plain8":
                                    nc.tensor.matmul(
                                        ps[c2][:, r, :],
                                        t_w1c.ap()[:, ap_i, 0, c2 * 128:(c2 + 1) * 128],
                                        bass.AP(tensor=ydf,
                                                offset=(h0 + r) * YM + 2 * ap_i,
                                                ap=[[YP, 128], [1, WOU]]),
                                        start=st_, stop=sp_)
                                    continue
                                # (a, a+NAP) tap pairing: k-pair dim stride NAP
                                # (20B) — a 1B stride hangs the real PE.
                                rhs = bass.AP(
                                    tensor=ydf,
                                    offset=(h0 + r) * YM + ap_i,
                                    ap=[[YP, 128], [NAP, 2], [1, WOU]],
                                )
                                nc.tensor.matmul(
                                    ps[c2][:, r, :],
                                    lhsT,
                                    rhs,
                                    start=st_, stop=sp_,
                                    perf_mode=DR,
                                )
                    if stage == "mm":
                        continue
                    # evac: relu(psum/WSC + b1) -> x fp8, layout [(ph,pw),(eh,ew)]
                    for c2 in range(2):
                        pflat = ps[c2][:]
                        r = 0
                        while r < hb:
                            h = h0 + r
                            ph, eh = h % 10, h // 10
                            nr = min(hb - r, 10 - ph)
                            nc.scalar.activation(
                                out=bass.AP(
                                    tensor=t_x,
                                    offset=c2 * PQ * EF + ph * 10 * EF + eh * 12,
                                    ap=[[XP, 128], [10 * EF, nr], [EF, 10], [1, 12]],
                                ),
                                in_=pflat[:, r:r + nr, 0:WOU].rearrange(
                                    "p r (e q) -> p r q e", q=10),
                                func=AF.Relu,
                                bias=t_b1.ap()[:, c2:c2 + 1],
                                scale=1.0 / WSC,
                            )
                            r += nr

                if stage in ("conv", "mm"):
                    continue

                # ---- primary caps conv (fp8 DoubleRow, both ch halves) ----
                pp = pps.tile([EF, D2], F32, tag="pp", name="pp")
                for pq in range(PQ):
                    nc.tensor.matmul(
                        pp[:],
                        t_x.ap()[:, :, pq, :],
                        t_wpq.ap()[:, :, pq, :],
                        start=(pq == 0),
                        stop=(pq == PQ - 1),
                        perf_mode=DR,
                    )

                # ---- squash (fold /WSC for prim scale and /CWS for caps_w) ----
                # u_eff = psum' * l2' / (CWS*(WSC^2 + l2'/WSC^2... ) see host notes:
                # p = psum'/WSC, l2t = l2'/WSC^2, u = p*l2t/((1+l2t)sqrt(l2t))
                #   = psum' * l2' / ((WSC^2 + l2') * sqrt(l2'))
                # u_eff = u / CWS  ->  f2 = CWS*l2' + CWS*WSC^2
                nc.vector.tensor_tensor(out=t_pb.ap(), in0=pp[:], in1=t_bpx.ap(),
                                        op=ALU.add)
                nc.vector.tensor_tensor(out=t_sq.ap(), in0=t_pb.ap(), in1=t_pb.ap(),
                                        op=ALU.mult)
                nc.vector.tensor_reduce(
                    out=t_l2.ap(),
                    in_=t_sq.ap().rearrange("p (g k) -> p g k", g=G),
                    axis=mybir.AxisListType.X, op=ALU.add,
                )
                nc.scalar.sqrt(t_f1.ap(), t_l2.ap())
                nc.vector.tensor_scalar(
                    out=t_f2.ap(), in0=t_l2.ap(),
                    scalar1=CWS, scalar2=CWS * WSC * WSC,
                    op0=ALU.mult, op1=ALU.add,
                )
                nc.vector.tensor_tensor(out=t_f3.ap(), in0=t_f1.ap(), in1=t_f2.ap(),
                                        op=ALU.mult)
                nc.vector.reciprocal(t_f4.ap(), t_f3.ap())
                nc.vector.tensor_tensor(out=t_f5.ap(), in0=t_l2.ap(), in1=t_f4.ap(),
                                        op=ALU.mult)
                nc.vector.tensor_tensor(
                    out=t_u.ap()[:, im, :, :],
                    in0=t_pb.ap().rearrange("p (g k) -> p g k", g=G),
                    in1=t_f5.ap().unsqueeze(2).broadcast_to([EF, G, NK]),
                    op=ALU.mult,
                )

                if stage == "prim":
                    continue

                # ---- capsule prediction MACs (DVE only; walrus rejects any
                # per-partition-scalar op on Pool) ----
                for eng, g_lo, g_hi in ((nc.vector, 0, DVE_G),
                                        (nc.vector, DVE_G, G)):
                    for g in range(g_lo, g_hi):
                        dst = up_slice(g, im)
                        cwg = t_cw.ap()[:, g, :, :]
                        for k in range(NK):
                            usc = t_u.ap()[:, im:im + 1, g:g + 1, k:k + 1]
                            if k == 0:
                                eng.tensor_scalar(
                                    out=dst, in0=cwg[:, k, :],
                                    scalar1=usc, scalar2=None, op0=ALU.mult,
                                )
                            else:
                                eng.scalar_tensor_tensor(
                                    out=dst, in0=cwg[:, k, :], scalar=usc,
                                    in1=dst, op0=ALU.mult, op1=ALU.add,
                                )

                # ---- upT transposes (PE) + evac (DVE) ----
                # (the Tile scheduler orders these after the producing MACs)
                for g0 in range(0, G, 4):
                    tt = tps.tile([OD, 4, EF], BF16, tag="t", name="tt")
                    for j in range(4):
                        nc.tensor.matmul(
                            tt[:, j, :],
                            up_slice(g0 + j, im),
                            t_idb.ap()[0:EF, 0:EF],
                            is_transpose=True,
                            start=(j == 0), stop=(j == 3),
                        )
                    nc.vector.tensor_copy(
                        out=t_upT.ap()[:, im, g0:g0 + 4, :],
                        in_=tt[:],
                    )

            if stage in ("dma", "mm", "conv", "prim", "caps"):
                nc.sync.dma_start(out=d_out.ap(), in_=t_lg.ap())
                return nc

        # ================= routing tail =================
        # bank budget: rps 4 tags + sps 1 + vps 2 + wps 1 = 8 of 8
        with ExitStack() as ctx:
            rps = ctx.enter_context(tc.tile_pool(name="rps", bufs=1, space="PSUM"))
            sps = ctx.enter_context(tc.tile_pool(name="sps", bufs=1, space="PSUM"))
            vps = ctx.enter_context(tc.tile_pool(name="vps", bufs=1, space="PSUM"))
            wps = ctx.enter_context(tc.tile_pool(name="wps", bufs=1, space="PSUM"))

            # wcf broadcast to IMG partitions (K=1 matmul) + evac
            wcb = wps.tile([IMG, 26 * FD + 26], F32, tag="w", name="wcb")
            nc.tensor.matmul(wcb[:], t_o4.ap(), t_wcf.ap(), start=True, stop=True)
            nc.vector.tensor_copy(out=t_wcb.ap(), in_=wcb[:])

            for it in range(NITER):
                if it > 0:
                    # bb += sum_d upT . v   (per-(g,im) matmuls vs block-diag v)
                    rp = [rps.tile([EF, G * NCL], F32, tag=f"r{im}", name=f"rp{im}")
                          for im in range(IMG)]
                    for im in range(IMG):
                        for g in range(G):
                            nc.tensor.matmul(
                                rp[im][:, g * NCL:(g + 1) * NCL],
                                t_upT.ap()[:, im, g, :],
                                t_vbd.ap()[:, im, :],
                                start=(g == 0), stop=(g == G - 1),
                            )
                    for im in range(IMG):
                        nc.vector.tensor_tensor(
                            out=t_bb.ap()[:, im, :, :].rearrange("p g o -> p (g o)"),
                            in0=t_bb.ap()[:, im, :, :].rearrange("p g o -> p (g o)"),
                            in1=rp[im][:],
                            op=ALU.add,
                        )
                    # c = softmax over classes
                    nc.scalar.activation(
                        out=t_ce.ap().rearrange("p i g o -> p (i g o)"),
                        in_=t_bb.ap().rearrange("p i g o -> p (i g o)"),
                        func=AF.Exp,
                    )
                    nc.vector.tensor_reduce(
                        out=t_cs.ap(),
                        in_=t_ce.ap().rearrange("p i g o -> p (i g) o"),
                        axis=mybir.AxisListType.X, op=ALU.add,
                    )
                    nc.vector.reciprocal(t_cr.ap(), t_cs.ap())
                    nc.vector.tensor_tensor(
                        out=t_cc.ap().rearrange("p i g o -> p (i g) o"),
                        in0=t_ce.ap().rearrange("p i g o -> p (i g) o"),
                        in1=t_cr.ap().unsqueeze(2).broadcast_to([EF, IMG * G, NCL]),
                        op=ALU.mult,
                    )

                # s[od, o] = sum_i c[i,o] up[i,od]  (diag blocks are s)
                sp = sps.tile([OD, IMG, NCL], F32, tag="s", name="sp")
                first, last = (0, 0), (IMG - 1, G - 1)
                for im in range(IMG):
                    for g in range(G):
                        mov = (t_c0.ap()[:, g, :] if it == 0
                               else t_cc.ap()[:, im, g, :])
                        nc.tensor.matmul(
                            sp[:, im, :],
                            up_slice(g, im),
                            mov,
                            start=((im, g) == first), stop=((im, g) == last),
                        )
                # sT = diag-extract via mask, then transpose to [IMG, OD]
                nc.vector.tensor_tensor(
                    out=t_sm.ap(), in0=sp[:],
                    in1=t_mask.ap().unsqueeze(1).broadcast_to([OD, IMG, NCL]),
                    op=ALU.mult,
                )
                nc.vector.tensor_reduce(
                    out=t_sT.ap(), in_=t_sm.ap(), axis=mybir.AxisListType.X,
                    op=ALU.add,
                )
                st = vps.tile([IMG, OD], BF16, tag="v", name="st")
                nc.tensor.matmul(st[:], t_sT.ap(), t_idb.ap()[0:OD, 0:OD],
                                 is_transpose=True, start=True, stop=True)
                nc.vector.tensor_copy(out=t_sf.ap(), in_=st[:])
                # squash on [IMG, OD]
                nc.vector.tensor_tensor(out=t_q1.ap(), in0=t_sf.ap(),
                                        in1=t_sf.ap(), op=ALU.mult)
                nc.vector.tensor_reduce(
                    out=t_q2.ap(),
                    in_=t_q1.ap().rearrange("p (o d) -> p o d", o=NCL),
                    axis=mybir.AxisListType.X, op=ALU.add,
                )
                nc.scalar.sqrt(t_q3.ap(), t_q2.ap())
                nc.vector.tensor_scalar(out=t_q4.ap(), in0=t_q2.ap(),
                                        scalar1=1.0, scalar2=None, op0=ALU.add)
                nc.vector.tensor_tensor(out=t_q5.ap(), in0=t_q3.ap(),
                                        in1=t_q4.ap(), op=ALU.mult)
                nc.vector.reciprocal(t_q6.ap(), t_q5.ap())
                # squash factor = l2/((1+l2)*sqrt(l2))
                nc.vector.tensor_tensor(out=t_q5.ap(), in0=t_q2.ap(),
                                        in1=t_q6.ap(), op=ALU.mult)
                nc.vector.tensor_tensor(
                    out=t_v.ap().rearrange("p (o d) -> p o d", o=NCL),
                    in0=t_sf.ap().rearrange("p (o d) -> p o d", o=NCL),
                    in1=t_q5.ap().unsqueeze(2).broadcast_to([IMG, NCL, FD]),
                    op=ALU.mult,
                )
                if it == NITER - 1:
                    nc.vector.tensor_tensor(
                        out=t_vf.ap().rearrange("p (o d) -> p o d", o=NCL),
                        in0=t_sf.ap().rearrange("p (o d) -> p o d", o=NCL),
                        in1=t_q5.ap().unsqueeze(2).broadcast_to([IMG, NCL, FD]),
                        op=ALU.mult,
                    )
                else:
                    # vT + block-diag expansion for next iteration's bb-dot
                    vt = vps.tile([OD, IMG], BF16, tag="w", name="vt")
                    nc.tensor.matmul(vt[:], t_v.ap(), t_idb.ap()[0:IMG, 0:IMG],
                                     is_transpose=True, start=True, stop=True)
                    nc.vector.tensor_copy(out=t_vT.ap(), in_=vt[:])
                    nc.vector.tensor_tensor(
                        out=t_vbd.ap(),
                        in0=t_vT.ap().unsqueeze(2).broadcast_to([OD, IMG, NCL]),
                        in1=t_mask.ap().unsqueeze(1).broadcast_to([OD, IMG, NCL]),
                        op=ALU.mult,
                    )

            # ================= class head (free-dim log_softmax) =================
            # hm[im, (c,d)] = vf[im, (o(c?),d)] -- contract d per (o,c)
            for o in range(NCL):
                nc.vector.tensor_tensor(
                    out=t_hm.ap().rearrange("p (c d) -> p c d", c=26),
                    in0=t_vf.ap()[:, o * FD:(o + 1) * FD].unsqueeze(1)
                        .broadcast_to([IMG, 26, FD]),
                    in1=t_wcb.ap()[:, 0:26 * FD].rearrange("p (c d) -> p c d", c=26),
                    op=ALU.mult,
                )
                nc.vector.tensor_reduce(
                    out=t_li.ap()[:, o, :],
                    in_=t_hm.ap().rearrange("p (c d) -> p c d", c=26),
                    axis=mybir.AxisListType.X, op=ALU.add,
                )
            nc.vector.tensor_tensor(
                out=t_li.ap(),
                in0=t_li.ap(),
                in1=t_wcb.ap()[:, 26 * FD:].unsqueeze(1).broadcast_to([IMG, NCL, 26]),
                op=ALU.add,
            )
            nc.scalar.activation(out=t_ee.ap(), in_=t_li.ap(), func=AF.Exp)
            nc.vector.tensor_reduce(out=t_es.ap(), in_=t_ee.ap(),
                                    axis=mybir.AxisListType.X, op=ALU.add)
            nc.scalar.activation(out=t_ln.ap(), in_=t_es.ap(), func=AF.Ln)
            nc.vector.tensor_tensor(
                out=t_lg.ap().rearrange("p (o c) -> p o c", o=NCL),
                in0=t_li.ap(),
                in1=t_ln.ap().unsqueeze(2).broadcast_to([IMG, NCL, 26]),
                op=ALU.subtract,
            )
            nc.sync.dma_start(out=d_out.ap(), in_=t_lg.ap())

    return nc


def _legalize_waits(nc, max_waits=1):
    """Split multi-sem waits into single-wait NOP prefixes on the same engine."""
    n = 0
    for f in nc.m.functions:
        for b in f.blocks:
            il = b.instructions
            out = []
            for inst in il:
                si = inst.sync_info
                if si is not None and si.on_wait and len(si.on_wait) > max_waits:
                    waits = list(si.on_wait)
                    for w in waits[:-max_waits]:
                        n += 1
                        nop = mybir.InstNoOp(
                            name=f"I-waitfix-{n}", ins=[], outs=[],
                            engine=inst.engine,
                            sync_info=mybir.SyncInfo(on_wait=[w], on_update=[]),
                        )
                        nc.inst_map[nop.name] = nop
                        out.append(nop)
                    inst.sync_info = mybir.SyncInfo(
                        on_wait=waits[-max_waits:], on_update=list(si.on_update)
                    )
                out.append(inst)
            il[:] = out
    return n


_CACHE = {}


def build_nc():
    if "nc" not in _CACHE:
        nc = bass.Bass("TRN2", target_bir_lowering=False, debug=False)
        _emit(nc)
        _legalize_waits(nc)
        _CACHE["nc"] = nc
    return _CACHE["nc"]


def host_prep(inputs):
    """Preprocess weights on host into device layouts (shared by all cores)."""
    w1 = np.asarray(inputs["conv1_w"], np.float32)      # (256,1,30,160)
    # w1c[(s,i), ap, j, c] = 16 * w1[c, 0, i, 4*(ap + NAP*j)+s]
    w1r = (w1[:, 0] * WSC).reshape(C1, KH, 2, NAP, S)   # c, i, j, ap, s
    w1c = np.zeros((128, NAP, 2, C1), np.float32)
    w1c[:S * KH] = w1r.transpose(4, 1, 3, 2, 0).reshape(S * KH, NAP, 2, C1)
    w1c = np.ascontiguousarray(w1c).astype(E4NP)
    wp = np.asarray(inputs["prim_w"], np.float32)       # (256,256,10,10)
    # wpq[cl, j, pq, co] = 16 * wp[co, j*128+cl, ph, pw], pq = ph*10+pw
    wpq = np.ascontiguousarray(
        (wp * WSC).reshape(D2, 2, 128, PQ).transpose(2, 1, 3, 0)).astype(E4NP)
    cw = np.asarray(inputs["caps_w"], np.float32)       # (3456, 8, 80)
    # capsule i = g*108 + ef
    cwq = np.ascontiguousarray(
        (cw * CWS).reshape(G, EF, NK, OD).transpose(1, 0, 2, 3)).astype(E4NP)
    br = np.asarray(inputs["b_route"], np.float32)      # (3456, 5)
    br_g = br.reshape(G, EF, NCL).transpose(1, 0, 2)    # (EF, G, 5)
    bb0 = np.ascontiguousarray(
        np.broadcast_to(br_g[:, None], (EF, IMG, G, NCL))).astype(BFNP)
    e = np.exp(br - br.max(axis=1, keepdims=True))
    c0 = (e / e.sum(axis=1, keepdims=True)).astype(np.float32)
    c0 = np.ascontiguousarray(
        c0.reshape(G, EF, NCL).transpose(1, 0, 2)).astype(BFNP)
    mask = np.ascontiguousarray(np.repeat(np.eye(NCL, dtype=np.float32), FD,
                                          axis=0)).astype(BFNP)
    eye = np.eye(128, dtype=np.float32)
    pred_w = np.asarray(inputs["pred_w"], np.float32)
    eos_w = np.asarray(inputs["eos_w"], np.float32)
    pred_b = np.asarray(inputs["pred_b"], np.float32)
    eos_b = np.asarray(inputs["eos_b"], np.float32)
    wc = np.concatenate([pred_w, eos_w], 0)             # (26,16)
    bc = np.concatenate([pred_b, eos_b], 0)             # (26,)
    wcf = np.concatenate([wc.reshape(-1), bc])[None, :]  # (1, 442)
    return {
        "w1c": w1c,
        "b1": np.ascontiguousarray(
            np.asarray(inputs["conv1_b"], np.float32).reshape(2, 128).T),
        "wpq": wpq,
        "bp16": np.asarray(inputs["prim_b"], np.float32) * WSC,
        "cw": cwq,
        "bb0": bb0,
        "c0": c0,
        "mask": mask,
        "idb": np.ascontiguousarray(eye).astype(BFNP),
        "idf": np.ascontiguousarray(eye),
        "wcf": np.ascontiguousarray(wcf),
    }


def make_ydf(img4):
    """img4: (IMG, 120, 640) f32 -> (IMG, 120, HOU, YM) fp8 toeplitz."""
    q = img4.astype(E4NP).astype(np.float32)  # quantize once, then gather
    out = np.zeros((IMG, 128, HOU, YM), np.float32)
    for im in range(IMG):
        v = q[im].reshape(HI, YM, S)                    # (h, m, s)
        for s in range(S):
            arr = np.ascontiguousarray(v[:, :, s])      # (120, 160)
            st = arr.strides
            win = np.lib.stride_tricks.as_strided(
                arr, shape=(KH, HOU, YM), strides=(st[0], st[0], st[1]))
            out[im, s * KH:(s + 1) * KH] = win
    return np.ascontiguousarray(out).astype(E4NP)


def make_in_maps(inputs):
    shared = host_prep(inputs)
    x = np.asarray(inputs["input"], np.float32)  # (32,1,120,640)
    in_maps = []
    for c in range(N_CORES):
        m = dict(shared)
        m["ydf"] = make_ydf(x[c * IMG:(c + 1) * IMG, 0])
        in_maps.append(m)
    return in_maps


def _get_runner():
    """Build (once) a jitted shard_map over the bass_exec custom call."""
    if "runner" in _CACHE:
        return _CACHE["runner"]
    import jax
    import concourse.mybir as _mybir
    from jax.sharding import Mesh, PartitionSpec
    from jax.experimental.shard_map import shard_map
    from concourse import bass2jax
    bass2jax.install_neuronx_cc_hook()
    nc = build_nc()
    pname = nc.partition_id_tensor.name if nc.partition_id_tensor else None
    in_names, out_names, out_avals, zero_outs = [], [], [], []
    for alloc in nc.m.functions[0].allocations:
        if not isinstance(alloc, _mybir.MemoryLocationSet):
            continue
        name = alloc.memorylocations[0].name
        if alloc.kind == "ExternalInput":
            if name != pname:
                in_names.append(name)
        elif alloc.kind == "ExternalOutput":
            out_names.append(name)
            shape = tuple(alloc.tensor_shape)
            dtype = _mybir.dt.np(alloc.dtype)
            out_avals.append(jax.core.ShapedArray(shape, dtype))
            zero_outs.append(np.zeros(shape, dtype))
    n_params = len(in_names)
    all_names = in_names + out_names
    if pname is not None:
        all_names = all_names + [pname]

    def _body(*args):
        operands = list(args)
        if pname is not None:
            operands.append(bass2jax.partition_id_tensor())
        outs = bass2jax._bass_exec_p.bind(
            *operands,
            out_avals=tuple(out_avals),
            in_names=tuple(all_names),
            out_names=tuple(out_names),
            lowering_input_output_aliases=(),
            sim_require_finite=True,
            sim_require_nnan=True,
            nc=nc,
        )
        return tuple(outs)

    devices = jax.devices()[:N_CORES]
    mesh = Mesh(np.asarray(devices), ("core",))
    n_outs = len(out_names)
    sharded = jax.jit(
        shard_map(_body, mesh=mesh,
                  in_specs=(PartitionSpec("core"),) * (n_params + n_outs),
                  out_specs=(PartitionSpec("core"),) * n_outs,
                  check_rep=False),
        donate_argnums=tuple(range(n_params, n_params + n_outs)),
        keep_unused=True,
    )
    _CACHE["runner"] = (sharded, in_names, out_names, zero_outs)
    return _CACHE["runner"]


def run_sharded(in_maps):
    sharded, in_names, out_names, zero_outs = _get_runner()
    concat_in = [
        np.concatenate([np.asarray(in_maps[c][n]) for c in range(N_CORES)], axis=0)
        for n in in_names
    ]
    concat_zero = [np.concatenate([z] * N_CORES, axis=0) for z in zero_outs]
    outs = sharded(*concat_in, *concat_zero)
    res = []
    for c in range(N_CORES):
        m = {}
        for i, n in enumerate(out_names):
            arr = np.asarray(outs[i])
            per = arr.shape[0] // N_CORES
            m[n] = arr[c * per:(c + 1) * per]
        res.append(m)
    return res


def kernel(**inputs):
    in_maps = make_in_maps(inputs)
    res = run_sharded(in_maps)
    out = np.concatenate(
        [np.asarray(r["out"], np.float32).reshape(IMG, NCL, 26) for r in res],
        axis=0,
    )
    return out


# revision 37
# speedup vs baseline: 1.0057x; 1.0057x over previous
"""CapsNet forward pass on 8 Trainium2 NeuronCores (pure data parallelism).

Per core (4 images), all heavy math in fp8 DoubleRow on the PE:
  conv1 (256x1x30x160, stride (1,4)) as 20 accumulated fp8-DoubleRow matmuls
  per psum row (width-tap pair (a, a+20) per instruction -- the k-pair dim
  needs a multi-byte stride, a 1B stride hangs the real PE; K=(s,i)=120 of
  128 partitions x 2 k-tiles = 240 taps/step at 0.5 cyc/row); the Toeplitz
  operand is built on the HOST and shipped as one DMA per image.
  Relu+bias+fp8-quantize evac on ACT directly into the primary-conv
  stationary layout.
  -> primary caps conv (256x256x10x10, stride 10) as 100 fp8-DoubleRow
  matmuls (both 128-channel halves per instruction), output [spatial, ch].
  -> squash (+ fp8/bias scale folding) on DVE -> capsule prediction as
  per-partition-scalar MACs on DVE (walrus rejects TensorScalarPtr on Pool),
  bf16 accumulation.
  -> agreement routing with all contractions on the PE: per-(g,im) matmuls
  against a block-diagonal v operand for the b-updates, c-weighted capsule
  sums, and PE transposes for layout changes; only softmax/squash pointwise
  work stays on DVE/ACT (bf16).
  -> class head via a K=1 broadcast matmul + DVE contraction + free-dim
  log_softmax. Host only shards/stacks.
"""

import numpy as np
import ml_dtypes
from contextlib import ExitStack

import concourse.bass as bass
import concourse.tile as tile
import concourse.mybir as mybir
from concourse.bass_utils import run_bass_kernel_spmd  # noqa: F401  (kept for parity)

F32 = mybir.dt.float32
BF16 = mybir.dt.bfloat16
FP8 = mybir.dt.float8e4
AF = mybir.ActivationFunctionType
ALU = mybir.AluOpType
DR = mybir.MatmulPerfMode.DoubleRow

E4NP = ml_dtypes.float8_e4m3
BFNP = ml_dtypes.bfloat16

# Problem constants
N_CORES = 8
IMG = 4              # images per core
HI, WI = 120, 640    # input image
KH, KW = 30, 160     # conv1 kernel
S = 4                # conv1 width stride (phase count)
A = KW // S          # 40 width taps per phase
NAP = A // 2         # 20 DoubleRow tap-pair steps
HOU, WOU = 90, 120   # conv1 output rows/cols actually consumed by prim conv
HB = 4               # conv1 row-block (psum bank = 2 DoubleRow matmuls)
C1 = 256
PQ = 100             # prim kernel positions (10x10)
EF = 108             # prim output spatial (9*12)
D2 = 256             # prim output channels
G = 32               # capsule groups
NK = 8               # capsule input dim
OD = 80              # 5 classes * 16
NCL, FD = 5, 16
NITER = 4            # initial softmax round + 3 routing iterations
YM = 160             # toeplitz m extent
XP = 2 * PQ * EF     # x free pitch (21600)
WSC = 16.0           # fp8 weight scale for conv1/prim
CWS = 16.0           # fp8 scale for caps weights

DVE_G = 20           # capsule groups on DVE; rest on Pool(gpsimd)


def _emit(nc):
    import os
    stage = os.environ.get("K_STAGE", "all")  # dma|mm|conv|prim|caps|all
    # ---- DRAM I/O ----
    d_ydf = nc.dram_tensor("ydf", [IMG, 128, HOU, YM], FP8, kind="ExternalInput")
    d_w1c = nc.dram_tensor("w1c", [128, NAP, 2, C1], FP8, kind="ExternalInput")
    d_b1 = nc.dram_tensor("b1", [128, 2], F32, kind="ExternalInput")
    d_wpq = nc.dram_tensor("wpq", [128, 2, PQ, D2], FP8, kind="ExternalInput")
    d_bp16 = nc.dram_tensor("bp16", [D2], F32, kind="ExternalInput")
    d_cw = nc.dram_tensor("cw", [EF, G, NK, OD], FP8, kind="ExternalInput")
    d_bb0 = nc.dram_tensor("bb0", [EF, IMG, G, NCL], BF16, kind="ExternalInput")
    d_c0 = nc.dram_tensor("c0", [EF, G, NCL], BF16, kind="ExternalInput")
    d_mask = nc.dram_tensor("mask", [OD, NCL], BF16, kind="ExternalInput")
    d_idb = nc.dram_tensor("idb", [128, 128], BF16, kind="ExternalInput")
    d_idf = nc.dram_tensor("idf", [128, 128], F32, kind="ExternalInput")
    d_wcf = nc.dram_tensor("wcf", [1, 26 * FD + 26], F32, kind="ExternalInput")
    d_out = nc.dram_tensor("out", [IMG, NCL * 26], F32, kind="ExternalOutput")

    # ---- persistent SBUF ----
    t_ydf = [nc.alloc_sbuf_tensor(f"s_ydf{i}", [128, HOU, YM], FP8)
             for i in range(2)]
    t_w1c = nc.alloc_sbuf_tensor("s_w1c", [128, NAP, 2, C1], FP8)
    t_b1 = nc.alloc_sbuf_tensor("s_b1", [128, 2], F32)
    t_x = nc.alloc_sbuf_tensor("s_x", [128, 2, PQ, EF], FP8)
    t_wpq = nc.alloc_sbuf_tensor("s_wpq", [128, 2, PQ, D2], FP8)
    t_bpx = nc.alloc_sbuf_tensor("s_bpx", [EF, D2], F32)
    t_cw = nc.alloc_sbuf_tensor("s_cw", [EF, G, NK, OD], FP8)
    t_pb = nc.alloc_sbuf_tensor("s_pb", [EF, D2], F32)
    t_sq = nc.alloc_sbuf_tensor("s_sq", [EF, D2], F32)
    t_l2 = nc.alloc_sbuf_tensor("s_l2", [EF, G], F32)
    t_f1 = nc.alloc_sbuf_tensor("s_f1", [EF, G], F32)
    t_f2 = nc.alloc_sbuf_tensor("s_f2", [EF, G], F32)
    t_f3 = nc.alloc_sbuf_tensor("s_f3", [EF, G], F32)
    t_f4 = nc.alloc_sbuf_tensor("s_f4", [EF, G], F32)
    t_f5 = nc.alloc_sbuf_tensor("s_f5", [EF, G], F32)
    t_u = nc.alloc_sbuf_tensor("s_u", [EF, IMG, G, NK], F32)
    t_upA = nc.alloc_sbuf_tensor("s_upA", [EF, DVE_G, IMG, OD], BF16)
    t_upB = nc.alloc_sbuf_tensor("s_upB", [EF, G - DVE_G, IMG, OD], BF16)
    t_upT = nc.alloc_sbuf_tensor("s_upT", [OD, IMG, G, EF], BF16)
    t_bb = nc.alloc_sbuf_tensor("s_bb", [EF, IMG, G, NCL], BF16)
    t_ce = nc.alloc_sbuf_tensor("s_ce", [EF, IMG, G, NCL], BF16)
    t_cc = nc.alloc_sbuf_tensor("s_cc", [EF, IMG, G, NCL], BF16)
    t_cs = nc.alloc_sbuf_tensor("s_cs", [EF, IMG * G], F32)
    t_cr = nc.alloc_sbuf_tensor("s_cr", [EF, IMG * G], F32)
    t_c0 = nc.alloc_sbuf_tensor("s_c0", [EF, G, NCL], BF16)
    t_mask = nc.alloc_sbuf_tensor("s_mask", [OD, NCL], BF16)
    t_idb = nc.alloc_sbuf_tensor("s_idb", [128, 128], BF16)
    t_idf = nc.alloc_sbuf_tensor("s_idf", [128, 128], F32)
    t_sm = nc.alloc_sbuf_tensor("s_sm", [OD, IMG, NCL], F32)
    t_sT = nc.alloc_sbuf_tensor("s_sT", [OD, IMG], BF16)
    t_sf = nc.alloc_sbuf_tensor("s_sf", [IMG, OD], F32)
    t_q1 = nc.alloc_sbuf_tensor("s_q1", [IMG, OD], F32)
    t_q2 = nc.alloc_sbuf_tensor("s_q2", [IMG, NCL], F32)
    t_q3 = nc.alloc_sbuf_tensor("s_q3", [IMG, NCL], F32)
    t_q4 = nc.alloc_sbuf_tensor("s_q4", [IMG, NCL], F32)
    t_q5 = nc.alloc_sbuf_tensor("s_q5", [IMG, NCL], F32)
    t_q6 = nc.alloc_sbuf_tensor("s_q6", [IMG, NCL], F32)
    t_v = nc.alloc_sbuf_tensor("s_v", [IMG, OD], BF16)
    t_vf = nc.alloc_sbuf_tensor("s_vf", [IMG, OD], F32)
    t_vT = nc.alloc_sbuf_tensor("s_vT", [OD, IMG], BF16)
    t_vbd = nc.alloc_sbuf_tensor("s_vbd", [OD, IMG, NCL], BF16)
    t_o4 = nc.alloc_sbuf_tensor("s_o4", [1, IMG], F32)
    t_z8 = nc.alloc_sbuf_tensor("s_z8", [EF, OD], BF16)
    t_wcf = nc.alloc_sbuf_tensor("s_wcf", [1, 26 * FD + 26], F32)
    t_wcb = nc.alloc_sbuf_tensor("s_wcb", [IMG, 26 * FD + 26], F32)
    t_hm = nc.alloc_sbuf_tensor("s_hm", [IMG, NCL, 26, FD], F32)
    t_li = nc.alloc_sbuf_tensor("s_li", [IMG, NCL, 26], F32)
    t_ee = nc.alloc_sbuf_tensor("s_ee", [IMG, NCL, 26], F32)
    t_es = nc.alloc_sbuf_tensor("s_es", [IMG, NCL], F32)
    t_ln = nc.alloc_sbuf_tensor("s_ln", [IMG, NCL], F32)
    t_lg = nc.alloc_sbuf_tensor("s_lg", [IMG, NCL * 26], F32)

    YP = HOU * YM          # ydf free pitch per partition (elements)

    def up_slice(g, im):
        t = t_upA if g < DVE_G else t_upB
        gg = g if g < DVE_G else g - DVE_G
        return t.ap()[:, gg, im, :]

    with tile.TileContext(nc) as tc, nc.allow_low_precision(reason="bf16 routing"):
        with ExitStack() as ctx:
            # bank budget: cps 2 tags x 2 bufs + pps 1 + tps 2 = 7 of 8
            cps = ctx.enter_context(tc.tile_pool(name="cps", bufs=2, space="PSUM"))
            pps = ctx.enter_context(tc.tile_pool(name="pps", bufs=1, space="PSUM"))
            tps = ctx.enter_context(tc.tile_pool(name="tps", bufs=2, space="PSUM"))

            # ---- constant loads (order matters: conv1(im0) deps first) ----
            nc.sync.dma_start(out=t_w1c.ap(), in_=d_w1c.ap())
            # ydf im0 in two chunks so conv1 can start early
            nc.sync.dma_start(out=t_ydf[0].ap()[:, 0:20, :], in_=d_ydf.ap()[0][:, 0:20, :])
            nc.sync.dma_start(out=t_b1.ap(), in_=d_b1.ap())
            nc.sync.dma_start(out=t_ydf[0].ap()[:, 20:HOU, :], in_=d_ydf.ap()[0][:, 20:HOU, :])
            nc.sync.dma_start(out=t_wpq.ap(), in_=d_wpq.ap())
            nc.sync.dma_start(
                out=t_bpx.ap(), in_=bass.AP(tensor=d_bp16, offset=0, ap=[[0, EF], [1, D2]])
            )
            nc.sync.dma_start(out=t_cw.ap(), in_=d_cw.ap())
            nc.sync.dma_start(out=t_bb.ap(), in_=d_bb0.ap())
            nc.sync.dma_start(out=t_c0.ap(), in_=d_c0.ap())
            nc.sync.dma_start(out=t_mask.ap(), in_=d_mask.ap())
            nc.sync.dma_start(out=t_idb.ap(), in_=d_idb.ap())
            nc.sync.dma_start(out=t_idf.ap(), in_=d_idf.ap())
            nc.sync.dma_start(out=t_wcf.ap(), in_=d_wcf.ap())
            nc.vector.memset(t_o4.ap(), 1.0)
            nc.vector.memset(t_lg.ap(), 0.0)
            nc.vector.memset(t_z8.ap(), 0.0)

            # ================= per-image pipeline =================
            for im in range(IMG):
                ydf = t_ydf[im % 2]
                if im + 1 < IMG:
                    nc.sync.dma_start(out=t_ydf[(im + 1) % 2].ap(),
                                      in_=d_ydf.ap()[im + 1])
                if stage == "dma":
                    continue

                # ---- conv1: 23 blocks of 4 rows (last: 2) ----
                nblk = (HOU + HB - 1) // HB
                for b in range(nblk):
                    h0 = b * HB
                    hb = min(HB, HOU - h0)
                    ps = [cps.tile([128, HB, WOU], F32, tag=f"c{c2}",
                                   name=f"ps{c2}")
                          for c2 in range(2)]
                    mm = os.environ.get("K_MM", "dr")
                    for ap_i in range(NAP):
                        for c2 in range(2):
                            lhsT = t_w1c.ap()[:, ap_i, :, c2 * 128:(c2 + 1) * 128]
                            for r in range(hb):
                                st_ = (ap_i == 0 and r == 0)
                                sp_ = (ap_i == NAP - 1 and r == hb - 1)
                                if mm == "plain8":
                                    nc.tensor.matmul(
                                        ps[c2][:, r, :],
                                        t_w1c.ap()[:, ap_i, 0, c2 * 128:(c2 + 1) * 128],
                                        bass.AP(tensor=ydf,
                                                offset=(h0 + r) * YM + 2 * ap_i,
                                                ap=[[YP, 128], [1, WOU]]),
                                        start=st_, stop=sp_)
                                    continue
                                # (a, a+NAP) tap pairing: k-pair dim stride NAP
                                # (20B) — a 1B stride hangs the real PE.
                                rhs = bass.AP(
                                    tensor=ydf,
                                    offset=(h0 + r) * YM + ap_i,
                                    ap=[[YP, 128], [NAP, 2], [1, WOU]],
                                )
                                nc.tensor.matmul(
                                    ps[c2][:, r, :],
                                    lhsT,
                                    rhs,
                                    start=st_, stop=sp_,
                                    perf_mode=DR,
                                )
                    if stage == "mm":
                        continue
                    # evac: relu(psum/WSC + b1) -> x fp8, layout [(ph,pw),(eh,ew)]
                    for c2 in range(2):
                        pflat = ps[c2][:]
                        r = 0
                        while r < hb:
                            h = h0 + r
                            ph, eh = h % 10, h // 10
                            nr = min(hb - r, 10 - ph)
                            nc.scalar.activation(
                                out=bass.AP(
                                    tensor=t_x,
                                    offset=c2 * PQ * EF + ph * 10 * EF + eh * 12,
                                    ap=[[XP, 128], [10 * EF, nr], [EF, 10], [1, 12]],
                                ),
                                in_=pflat[:, r:r + nr, 0:WOU].rearrange(
                                    "p r (e q) -> p r q e", q=10),
                                func=AF.Relu,
                                bias=t_b1.ap()[:, c2:c2 + 1],
                                scale=1.0 / WSC,
                            )
                            r += nr

                if stage in ("conv", "mm"):
                    continue

                # ---- primary caps conv (fp8 DoubleRow, both ch halves) ----
                pp = pps.tile([EF, D2], F32, tag="pp", name="pp")
                for pq in range(PQ):
                    nc.tensor.matmul(
                        pp[:],
                        t_x.ap()[:, :, pq, :],
                        t_wpq.ap()[:, :, pq, :],
                        start=(pq == 0),
                        stop=(pq == PQ - 1),
                        perf_mode=DR,
                    )

                # ---- squash (fold /WSC for prim scale and /CWS for caps_w) ----
                # u_eff = psum' * l2' / (CWS*(WSC^2 + l2'/WSC^2... ) see host notes:
                # p = psum'/WSC, l2t = l2'/WSC^2, u = p*l2t/((1+l2t)sqrt(l2t))
                #   = psum' * l2' / ((WSC^2 + l2') * sqrt(l2'))
                # u_eff = u / CWS  ->  f2 = CWS*l2' + CWS*WSC^2
                nc.vector.tensor_tensor(out=t_pb.ap(), in0=pp[:], in1=t_bpx.ap(),
                                        op=ALU.add)
                nc.vector.tensor_tensor(out=t_sq.ap(), in0=t_pb.ap(), in1=t_pb.ap(),
                                        op=ALU.mult)
                nc.vector.tensor_reduce(
                    out=t_l2.ap(),
                    in_=t_sq.ap().rearrange("p (g k) -> p g k", g=G),
                    axis=mybir.AxisListType.X, op=ALU.add,
                )
                nc.scalar.sqrt(t_f1.ap(), t_l2.ap())
                nc.vector.tensor_scalar(
                    out=t_f2.ap(), in0=t_l2.ap(),
                    scalar1=CWS, scalar2=CWS * WSC * WSC,
                    op0=ALU.mult, op1=ALU.add,
                )
                nc.vector.tensor_tensor(out=t_f3.ap(), in0=t_f1.ap(), in1=t_f2.ap(),
                                        op=ALU.mult)
                nc.vector.reciprocal(t_f4.ap(), t_f3.ap())
                nc.vector.tensor_tensor(out=t_f5.ap(), in0=t_l2.ap(), in1=t_f4.ap(),
                                        op=ALU.mult)
                nc.vector.tensor_tensor(
                    out=t_u.ap()[:, im, :, :],
                    in0=t_pb.ap().rearrange("p (g k) -> p g k", g=G),
                    in1=t_f5.ap().unsqueeze(2).broadcast_to([EF, G, NK]),
                    op=ALU.mult,
                )

                if stage == "prim":
                    continue

                # ---- capsule prediction MACs (DVE only; walrus rejects any
                # per-partition-scalar op on Pool) ----
                for eng, g_lo, g_hi in ((nc.vector, 0, DVE_G),
                                        (nc.vector, DVE_G, G)):
                    for g in range(g_lo, g_hi):
                        dst = up_slice(g, im)
                        cwg = t_cw.ap()[:, g, :, :]
                        for k in range(NK):
                            usc = t_u.ap()[:, im:im + 1, g:g + 1, k:k + 1]
                            if k == 0:
                                eng.tensor_scalar(
                                    out=dst, in0=cwg[:, k, :],
                                    scalar1=usc, scalar2=None, op0=ALU.mult,
                                )
                            else:
                                eng.scalar_tensor_tensor(
                                    out=dst, in0=cwg[:, k, :], scalar=usc,
                                    in1=dst, op0=ALU.mult, op1=ALU.add,
                                )

                # ---- upT transposes (PE) + evac (DVE) ----
                # (the Tile scheduler orders these after the producing MACs)
                for g0 in range(0, G, 8):
                    tt = tps.tile([OD, 8, EF], BF16, tag="t", name="tt")
                    for j in range(8):
                        nc.tensor.matmul(
                            tt[:, j, :],
                            up_slice(g0 + j, im),
                            t_idb.ap()[0:EF, 0:EF],
                            is_transpose=True,
                            start=(j == 0), stop=(j == 7),
                        )
                    nc.vector.tensor_copy(
                        out=t_upT.ap()[:, im, g0:g0 + 8, :],
                        in_=tt[:],
                    )

            if stage in ("dma", "mm", "conv", "prim", "caps"):
                nc.sync.dma_start(out=d_out.ap(), in_=t_lg.ap())
                return nc

        # ================= routing tail =================
        # bank budget: rps 4 tags + sps 1 + vps 2 + wps 1 = 8 of 8
        with ExitStack() as ctx:
            rps = ctx.enter_context(tc.tile_pool(name="rps", bufs=1, space="PSUM"))
            sps = ctx.enter_context(tc.tile_pool(name="sps", bufs=1, space="PSUM"))
            vps = ctx.enter_context(tc.tile_pool(name="vps", bufs=1, space="PSUM"))
            wps = ctx.enter_context(tc.tile_pool(name="wps", bufs=1, space="PSUM"))

            # wcf broadcast to IMG partitions (K=1 matmul) + evac
            wcb = wps.tile([IMG, 26 * FD + 26], F32, tag="w", name="wcb")
            nc.tensor.matmul(wcb[:], t_o4.ap(), t_wcf.ap(), start=True, stop=True)
            nc.vector.tensor_copy(out=t_wcb.ap(), in_=wcb[:])

            for it in range(NITER):
                if it > 0:
                    # bb += sum_d upT . v   (per-(g,im) matmuls vs block-diag v)
                    rp = [rps.tile([EF, G * NCL], F32, tag=f"r{im}", name=f"rp{im}")
                          for im in range(IMG)]
                    for im in range(IMG):
                        for g in range(G):
                            nc.tensor.matmul(
                                rp[im][:, g * NCL:(g + 1) * NCL],
                                t_upT.ap()[:, im, g, :],
                                t_vbd.ap()[:, im, :],
                                start=(g == 0), stop=(g == G - 1),
                            )
                    for im in range(IMG):
                        nc.vector.tensor_tensor(
                            out=t_bb.ap()[:, im, :, :].rearrange("p g o -> p (g o)"),
                            in0=t_bb.ap()[:, im, :, :].rearrange("p g o -> p (g o)"),
                            in1=rp[im][:],
                            op=ALU.add,
                        )
                    # c = softmax over classes
                    nc.scalar.activation(
                        out=t_ce.ap().rearrange("p i g o -> p (i g o)"),
                        in_=t_bb.ap().rearrange("p i g o -> p (i g o)"),
                        func=AF.Exp,
                    )
                    nc.vector.tensor_reduce(
                        out=t_cs.ap(),
                        in_=t_ce.ap().rearrange("p i g o -> p (i g) o"),
                        axis=mybir.AxisListType.X, op=ALU.add,
                    )
                    nc.vector.reciprocal(t_cr.ap(), t_cs.ap())
                    nc.vector.tensor_tensor(
                        out=t_cc.ap().rearrange("p i g o -> p (i g) o"),
                        in0=t_ce.ap().rearrange("p i g o -> p (i g) o"),
                        in1=t_cr.ap().unsqueeze(2).broadcast_to([EF, IMG * G, NCL]),
                        op=ALU.mult,
                    )

                # s[od, o] = sum_i c[i,o] up[i,od]  (diag blocks are s)
                sp = sps.tile([OD, IMG, NCL], F32, tag="s", name="sp")
                first, last = (0, 0), (IMG - 1, G - 1)
                for im in range(IMG):
                    for g in range(G):
                        mov = (t_c0.ap()[:, g, :] if it == 0
                               else t_cc.ap()[:, im, g, :])
                        nc.tensor.matmul(
                            sp[:, im, :],
                            up_slice(g, im),
                            mov,
                            start=((im, g) == first), stop=((im, g) == last),
                        )
                # sT = diag-extract via mask, then transpose to [IMG, OD]
                nc.vector.tensor_tensor(
                    out=t_sm.ap(), in0=sp[:],
                    in1=t_mask.ap().unsqueeze(1).broadcast_to([OD, IMG, NCL]),
                    op=ALU.mult,
                )
                nc.vector.tensor_reduce(
                    out=t_sT.ap(), in_=t_sm.ap(), axis=mybir.AxisListType.X,
                    op=ALU.add,
                )
                st = vps.tile([IMG, OD], BF16, tag="v", name="st")
                nc.tensor.matmul(st[:], t_sT.ap(), t_idb.ap()[0:OD, 0:OD],
                                 is_transpose=True, start=True, stop=True)
                nc.vector.tensor_copy(out=t_sf.ap(), in_=st[:])
                # squash on [IMG, OD]
                nc.vector.tensor_tensor(out=t_q1.ap(), in0=t_sf.ap(),
                                        in1=t_sf.ap(), op=ALU.mult)
                nc.vector.tensor_reduce(
                    out=t_q2.ap(),
                    in_=t_q1.ap().rearrange("p (o d) -> p o d", o=NCL),
                    axis=mybir.AxisListType.X, op=ALU.add,
                )
                nc.scalar.sqrt(t_q3.ap(), t_q2.ap())
                nc.vector.tensor_scalar(out=t_q4.ap(), in0=t_q2.ap(),
                                        scalar1=1.0, scalar2=None, op0=ALU.add)
                nc.vector.tensor_tensor(out=t_q5.ap(), in0=t_q3.ap(),
                                        in1=t_q4.ap(), op=ALU.mult)
                nc.vector.reciprocal(t_q6.ap(), t_q5.ap())
                # squash factor = l2/((1+l2)*sqrt(l2))
                nc.vector.tensor_tensor(out=t_q5.ap(), in0=t_q2.ap(),
                                        in1=t_q6.ap(), op=ALU.mult)
                nc.vector.tensor_tensor(
                    out=t_v.ap().rearrange("p (o d) -> p o d", o=NCL),
                    in0=t_sf.ap().rearrange("p (o d) -> p o d", o=NCL),
                    in1=t_q5.ap().unsqueeze(2).broadcast_to([IMG, NCL, FD]),
                    op=ALU.mult,
                )
                if it == NITER - 1:
                    nc.vector.tensor_tensor(
                        out=t_vf.ap().rearrange("p (o d) -> p o d", o=NCL),
                        in0=t_sf.ap().rearrange("p (o d) -> p o d", o=NCL),
                        in1=t_q5.ap().unsqueeze(2).broadcast_to([IMG, NCL, FD]),
                        op=ALU.mult,
                    )
                else:
                    # vT + block-diag expansion for next iteration's bb-dot
                    vt = vps.tile([OD, IMG], BF16, tag="w", name="vt")
                    nc.tensor.matmul(vt[:], t_v.ap(), t_idb.ap()[0:IMG, 0:IMG],
                                     is_transpose=True, start=True, stop=True)
                    nc.vector.tensor_copy(out=t_vT.ap(), in_=vt[:])
                    nc.vector.tensor_tensor(
                        out=t_vbd.ap(),
                        in0=t_vT.ap().unsqueeze(2).broadcast_to([OD, IMG, NCL]),
                        in1=t_mask.ap().unsqueeze(1).broadcast_to([OD, IMG, NCL]),
                        op=ALU.mult,
                    )

            # ================= class head (free-dim log_softmax) =================
            # hm[im, o, c, d] = vf[im, o, d] * wc[c, d]; contract d
            nc.vector.tensor_tensor(
                out=t_hm.ap(),
                in0=t_vf.ap().rearrange("p (o d) -> p o d", o=NCL)
                    .unsqueeze(2).broadcast_to([IMG, NCL, 26, FD]),
                in1=t_wcb.ap()[:, 0:26 * FD].rearrange("p (c d) -> p c d", c=26)
                    .unsqueeze(1).broadcast_to([IMG, NCL, 26, FD]),
                op=ALU.mult,
            )
            nc.vector.tensor_reduce(
                out=t_li.ap(),
                in_=t_hm.ap(),
                axis=mybir.AxisListType.X, op=ALU.add,
            )
            nc.vector.tensor_tensor(
                out=t_li.ap(),
                in0=t_li.ap(),
                in1=t_wcb.ap()[:, 26 * FD:].unsqueeze(1).broadcast_to([IMG, NCL, 26]),
                op=ALU.add,
            )
            nc.scalar.activation(out=t_ee.ap(), in_=t_li.ap(), func=AF.Exp)
            nc.vector.tensor_reduce(out=t_es.ap(), in_=t_ee.ap(),
                                    axis=mybir.AxisListType.X, op=ALU.add)
            nc.scalar.activation(out=t_ln.ap(), in_=t_es.ap(), func=AF.Ln)
            nc.vector.tensor_tensor(
                out=t_lg.ap().rearrange("p (o c) -> p o c", o=NCL),
                in0=t_li.ap(),
                in1=t_ln.ap().unsqueeze(2).broadcast_to([IMG, NCL, 26]),
                op=ALU.subtract,
            )
            nc.sync.dma_start(out=d_out.ap(), in_=t_lg.ap())

    return nc


def _legalize_waits(nc, max_waits=1):
    """Split multi-sem waits into single-wait NOP prefixes on the same engine."""
    n = 0
    for f in nc.m.functions:
        for b in f.blocks:
            il = b.instructions
            out = []
            for inst in il:
                si = inst.sync_info
                if si is not None and si.on_wait and len(si.on_wait) > max_waits:
                    waits = list(si.on_wait)
                    for w in waits[:-max_waits]:
                        n += 1
                        nop = mybir.InstNoOp(
                            name=f"I-waitfix-{n}", ins=[], outs=[],
                            engine=inst.engine,
                            sync_info=mybir.SyncInfo(on_wait=[w], on_update=[]),
                        )
                        nc.inst_map[nop.name] = nop
                        out.append(nop)
                    inst.sync_info = mybir.SyncInfo(
                        on_wait=waits[-max_waits:], on_update=list(si.on_update)
                    )
                out.append(inst)
            il[:] = out
    return n


_CACHE = {}


def build_nc():
    if "nc" not in _CACHE:
        nc = bass.Bass("TRN2", target_bir_lowering=False, debug=False)
        _emit(nc)
        _legalize_waits(nc)
        _CACHE["nc"] = nc
    return _CACHE["nc"]


def host_prep(inputs):
    """Preprocess weights on host into device layouts (shared by all cores)."""
    w1 = np.asarray(inputs["conv1_w"], np.float32)      # (256,1,30,160)
    # w1c[(s,i), ap, j, c] = 16 * w1[c, 0, i, 4*(ap + NAP*j)+s]
    w1r = (w1[:, 0] * WSC).reshape(C1, KH, 2, NAP, S)   # c, i, j, ap, s
    w1c = np.zeros((128, NAP, 2, C1), np.float32)
    w1c[:S * KH] = w1r.transpose(4, 1, 3, 2, 0).reshape(S * KH, NAP, 2, C1)
    w1c = np.ascontiguousarray(w1c).astype(E4NP)
    wp = np.asarray(inputs["prim_w"], np.float32)       # (256,256,10,10)
    # wpq[cl, j, pq, co] = 16 * wp[co, j*128+cl, ph, pw], pq = ph*10+pw
    wpq = np.ascontiguousarray(
        (wp * WSC).reshape(D2, 2, 128, PQ).transpose(2, 1, 3, 0)).astype(E4NP)
    cw = np.asarray(inputs["caps_w"], np.float32)       # (3456, 8, 80)
    # capsule i = g*108 + ef
    cwq = np.ascontiguousarray(
        (cw * CWS).reshape(G, EF, NK, OD).transpose(1, 0, 2, 3)).astype(E4NP)
    br = np.asarray(inputs["b_route"], np.float32)      # (3456, 5)
    br_g = br.reshape(G, EF, NCL).transpose(1, 0, 2)    # (EF, G, 5)
    bb0 = np.ascontiguousarray(
        np.broadcast_to(br_g[:, None], (EF, IMG, G, NCL))).astype(BFNP)
    e = np.exp(br - br.max(axis=1, keepdims=True))
    c0 = (e / e.sum(axis=1, keepdims=True)).astype(np.float32)
    c0 = np.ascontiguousarray(
        c0.reshape(G, EF, NCL).transpose(1, 0, 2)).astype(BFNP)
    mask = np.ascontiguousarray(np.repeat(np.eye(NCL, dtype=np.float32), FD,
                                          axis=0)).astype(BFNP)
    eye = np.eye(128, dtype=np.float32)
    pred_w = np.asarray(inputs["pred_w"], np.float32)
    eos_w = np.asarray(inputs["eos_w"], np.float32)
    pred_b = np.asarray(inputs["pred_b"], np.float32)
    eos_b = np.asarray(inputs["eos_b"], np.float32)
    wc = np.concatenate([pred_w, eos_w], 0)             # (26,16)
    bc = np.concatenate([pred_b, eos_b], 0)             # (26,)
    wcf = np.concatenate([wc.reshape(-1), bc])[None, :]  # (1, 442)
    return {
        "w1c": w1c,
        "b1": np.ascontiguousarray(
            np.asarray(inputs["conv1_b"], np.float32).reshape(2, 128).T),
        "wpq": wpq,
        "bp16": np.asarray(inputs["prim_b"], np.float32) * WSC,
        "cw": cwq,
        "bb0": bb0,
        "c0": c0,
        "mask": mask,
        "idb": np.ascontiguousarray(eye).astype(BFNP),
        "idf": np.ascontiguousarray(eye),
        "wcf": np.ascontiguousarray(wcf),
    }


def make_ydf(img4):
    """img4: (IMG, 120, 640) f32 -> (IMG, 120, HOU, YM) fp8 toeplitz."""
    q = img4.astype(E4NP).astype(np.float32)  # quantize once, then gather
    out = np.zeros((IMG, 128, HOU, YM), np.float32)
    for im in range(IMG):
        v = q[im].reshape(HI, YM, S)                    # (h, m, s)
        for s in range(S):
            arr = np.ascontiguousarray(v[:, :, s])      # (120, 160)
            st = arr.strides
            win = np.lib.stride_tricks.as_strided(
                arr, shape=(KH, HOU, YM), strides=(st[0], st[0], st[1]))
            out[im, s * KH:(s + 1) * KH] = win
    return np.ascontiguousarray(out).astype(E4NP)


def make_in_maps(inputs):
    shared = host_prep(inputs)
    x = np.asarray(inputs["input"], np.float32)  # (32,1,120,640)
    in_maps = []
    for c in range(N_CORES):
        m = dict(shared)
        m["ydf"] = make_ydf(x[c * IMG:(c + 1) * IMG, 0])
        in_maps.append(m)
    return in_maps


def _get_runner():
    """Build (once) a jitted shard_map over the bass_exec custom call."""
    if "runner" in _CACHE:
        return _CACHE["runner"]
    import jax
    import concourse.mybir as _mybir
    from jax.sharding import Mesh, PartitionSpec
    from jax.experimental.shard_map import shard_map
    from concourse import bass2jax
    bass2jax.install_neuronx_cc_hook()
    nc = build_nc()
    pname = nc.partition_id_tensor.name if nc.partition_id_tensor else None
    in_names, out_names, out_avals, zero_outs = [], [], [], []
    for alloc in nc.m.functions[0].allocations:
        if not isinstance(alloc, _mybir.MemoryLocationSet):
            continue
        name = alloc.memorylocations[0].name
        if alloc.kind == "ExternalInput":
            if name != pname:
                in_names.append(name)
        elif alloc.kind == "ExternalOutput":
            out_names.append(name)
            shape = tuple(alloc.tensor_shape)
            dtype = _mybir.dt.np(alloc.dtype)
            out_avals.append(jax.core.ShapedArray(shape, dtype))
            zero_outs.append(np.zeros(shape, dtype))
    n_params = len(in_names)
    all_names = in_names + out_names
    if pname is not None:
        all_names = all_names + [pname]

    def _body(*args):
        operands = list(args)
        if pname is not None:
            operands.append(bass2jax.partition_id_tensor())
        outs = bass2jax._bass_exec_p.bind(
            *operands,
            out_avals=tuple(out_avals),
            in_names=tuple(all_names),
            out_names=tuple(out_names),
            lowering_input_output_aliases=(),
            sim_require_finite=True,
            sim_require_nnan=True,
            nc=nc,
        )
        return tuple(outs)

    devices = jax.devices()[:N_CORES]
    mesh = Mesh(np.asarray(devices), ("core",))
    n_outs = len(out_names)
    sharded = jax.jit(
        shard_map(_body, mesh=mesh,
                  in_specs=(PartitionSpec("core"),) * (n_params + n_outs),
                  out_specs=(PartitionSpec("core"),) * n_outs,
                  check_rep=False),
        donate_argnums=tuple(range(n_params, n_params + n_outs)),
        keep_unused=True,
    )
    _CACHE["runner"] = (sharded, in_names, out_names, zero_outs)
    return _CACHE["runner"]


def run_sharded(in_maps):
    sharded, in_names, out_names, zero_outs = _get_runner()
    concat_in = [
        np.concatenate([np.asarray(in_maps[c][n]) for c in range(N_CORES)], axis=0)
        for n in in_names
    ]
    concat_zero = [np.concatenate([z] * N_CORES, axis=0) for z in zero_outs]
    outs = sharded(*concat_in, *concat_zero)
    res = []
    for c in range(N_CORES):
        m = {}
        for i, n in enumerate(out_names):
            arr = np.asarray(outs[i])
            per = arr.shape[0] // N_CORES
            m[n] = arr[c * per:(c + 1) * per]
        res.append(m)
    return res


def kernel(**inputs):
    in_maps = make_in_maps(inputs)
    res = run_sharded(in_maps)
    out = np.concatenate(
        [np.asarray(r["out"], np.float32).reshape(IMG, NCL, 26) for r in res],
        axis=0,
    )
    return out
